# revision 10
# baseline (speedup 1.0000x reference)
"""Trainium2 Bass kernel for nn_AttractorCore (all-pairs N-body + MLP dynamics).

Self-contained: hardcodes shapes/sharding. Accepts FULL inputs, returns FULL output.
Distribution: data-parallel over bodies. M=4136 bodies padded to 4224 = 33*128;
each of 8 cores owns 528 rows, split into asymmetric column halves (384/144).
Per iteration each core computes its [528 x 4224] interaction tile (f16 matmuls;
d^-3 via ACT abs_rsqrt + vector square/cube with the uniform mass folded into
G*DT), with a deep cross-iteration software pipeline: each half is updated
(boundary MLP + Hamiltonian via Tanh, damping, norm clip) and AllGathered
separately, and the next iteration's force pass starts on the 17 j-tiles fully
covered by the first gather while the second gather is still in flight — the PE
never drains, so it can hold its high p-state. j-major positions are rebuilt
with PE transposes into PSUM + wide DVE copies. Final stats epilogue on host.
"""
import os
import sys
import numpy as np
import ml_dtypes

sys.path.insert(0, "/opt/trn_rl_repo")

# ---- problem constants (hardcoded) ----
B, N, D, K = 4, 1024, 128, 10
G, DAMP, DT, ITERS, EPS, MAXNORM, HID = 1e-3, 0.95, 0.1, 50, 1e-6, 10.0, 64
M0 = B * N + B * K          # 4136 real bodies
NT = 33                     # j tiles of 128
MP = NT * 128               # 4224 padded bodies
NC = 8                      # cores
R = MP // NC                # 528 rows per core
RH0, RH1 = 384, 144         # asymmetric my-column halves
G2 = G * DT * 0.1           # gravity * dt * uniform body mass (folded)

# j-tiles fully inside some core's first-half (RH0) column range: their rbT/sqA
# data arrives with the first (A) AllGather. Same indices on every core.
ORDER = ([jt for jt in range(NT)
          if 128 * jt + 128 <= ((128 * jt) // R) * R + RH0] +
         [jt for jt in range(NT)
          if 128 * jt + 128 > ((128 * jt) // R) * R + RH0])

_PROGRAM_CACHE = {}


def _build_program(iters):
    import concourse.bacc as bacc
    import concourse.mybir as mybir
    import concourse.tile as tile

    dt = mybir.dt
    F32, F16, U16 = dt.float32, dt.float16, dt.uint16
    AF = mybir.ActivationFunctionType
    OP = mybir.AluOpType

    nc = bacc.Bacc("TRN2", target_bir_lowering=False, debug=False, num_devices=NC)

    def din(name, shape, dtype):
        return nc.dram_tensor(name, shape, dtype, kind="ExternalInput").ap()

    i_rT0 = din("rT0", [128, R], F32)
    i_vT0 = din("vT0", [128, R], F32)
    i_rbT0 = din("rbT0", [128, MP], F16)
    i_rfin0 = din("rfin0", [128, R], F16)
    i_sqA0 = din("sqA0", [2, MP], F16)        # row0 = ones, row1 = -0.5*sq
    i_sqBmy0 = din("sqBmy0", [2, R], F16)     # row0 = -0.5*sq(my), row1 = ones
    i_mask0 = din("mask0", [128, NT * RH0], U16)
    i_mask1 = din("mask1", [128, NT * RH1], U16)
    i_mnrow = din("mnrow", [1, R], F32)       # max-norm per my column (1e30 pads)
    i_mn2row = din("mn2row", [1, R], F32)     # cap for sq
    i_id128 = din("id128", [128, 128], F16)
    i_bw1a = din("bw1a", [128, HID], F16)
    i_bw1b = din("bw1b", [128, HID], F16)
    i_bw2 = din("bw2", [HID, HID], F16)
    i_bw3 = din("bw3", [HID, 128], F16)
    i_pw1a = din("pw1a", [128, 128], F16)
    i_pw1b = din("pw1b", [128, 128], F16)
    i_pw1Ta = din("pw1Ta", [128, 128], F16)
    i_pw1Tb = din("pw1Tb", [128, 128], F16)
    i_kw1a = din("kw1a", [128, 128], F16)
    i_kw1b = din("kw1b", [128, 128], F16)
    i_kw1Ta = din("kw1Ta", [128, 128], F16)
    i_kw1Tb = din("kw1Tb", [128, 128], F16)
    i_bb1c = din("bb1c", [HID, 1], F32)
    i_bb2c = din("bb2c", [HID, 1], F32)
    i_bb3c = din("bb3c", [128, 1], F32)
    i_pb1c = din("pb1c", [128, 2], F32)
    i_kb1c = din("kb1c", [128, 2], F32)
    i_pw2c = din("pw2c", [128, 2], F32)
    i_pw2cn = din("pw2cn", [128, 2], F32)
    i_kw2c = din("kw2c", [128, 2], F32)
    i_kw2cn = din("kw2cn", [128, 2], F32)

    o_r = nc.dram_tensor("r_out", [128, R], F32, kind="ExternalOutput").ap()

    HS = [slice(0, RH0), slice(RH0, R)]
    RHs = [RH0, RH1]
    OFFs = [0, RH0]
    LAG = 3
    INJ = 4     # order-index at which the previous half's update is injected
    GVs = [2, 4]

    with tile.TileContext(nc) as tc:
        with (
            tc.tile_pool(name="cpool", bufs=1) as cpool,
            tc.tile_pool(name="wpool", bufs=1) as wpool,
            tc.tile_pool(name="d2pool", bufs=3, space="PSUM") as d2pool,
            tc.tile_pool(name="fpool", bufs=1, space="PSUM") as fpool,
            tc.tile_pool(name="spool", bufs=1, space="PSUM") as spool,
            tc.tile_pool(name="rnpool", bufs=1, space="PSUM") as rnpool,
            tc.tile_pool(name="dpool", bufs=2, space="DRAM") as dpool,
        ):
            # ---- persistent tiles ----
            rT = cpool.tile([128, R], F32, tag="rT")
            vT = cpool.tile([128, R], F32, tag="vT")
            rbT = cpool.tile([128, MP], F16, tag="rbT")
            RN = cpool.tile([128, MP], F16, tag="RN")   # j-major, ORDER-indexed
            Ys = [cpool.tile([128, NT * RH0], F16, tag="Y0", name="Y0"),
                  cpool.tile([128, NT * RH1], F16, tag="Y1", name="Y1")]
            Us = [cpool.tile([128, NT * RH0], F16, tag="U0", name="U0"),
                  cpool.tile([128, NT * RH1], F16, tag="U1", name="U1")]
            Ms = [cpool.tile([128, NT * RH0], U16, tag="M0", name="M0"),
                  cpool.tile([128, NT * RH1], U16, tag="M1", name="M1")]
            sqA = cpool.tile([2, MP], F16, tag="sqA")
            sqBmy = cpool.tile([2, R], F16, tag="sqBmy")
            rfinb = cpool.tile([128, R], F16, tag="rfinb")
            mnrow = cpool.tile([1, R], F32, tag="mnrow")
            mn2row = cpool.tile([1, R], F32, tag="mn2row")
            id128 = cpool.tile([128, 128], F16, tag="id128")
            ones_col = cpool.tile([128, 1], F16, tag="ones_col")
            bw1a = cpool.tile([128, HID], F16, tag="bw1a")
            bw1b = cpool.tile([128, HID], F16, tag="bw1b")
            bw2 = cpool.tile([HID, HID], F16, tag="bw2")
            bw3 = cpool.tile([HID, 128], F16, tag="bw3")
            pw1a = cpool.tile([128, 128], F16, tag="pw1a")
            pw1b = cpool.tile([128, 128], F16, tag="pw1b")
            pw1Ta = cpool.tile([128, 128], F16, tag="pw1Ta")
            pw1Tb = cpool.tile([128, 128], F16, tag="pw1Tb")
            kw1a = cpool.tile([128, 128], F16, tag="kw1a")
            kw1b = cpool.tile([128, 128], F16, tag="kw1b")
            kw1Ta = cpool.tile([128, 128], F16, tag="kw1Ta")
            kw1Tb = cpool.tile([128, 128], F16, tag="kw1Tb")
            bb1c = cpool.tile([HID, 1], F32, tag="bb1c")
            bb2c = cpool.tile([HID, 1], F32, tag="bb2c")
            bb3c = cpool.tile([128, 1], F32, tag="bb3c")
            pb1c = cpool.tile([128, 2], F32, tag="pb1c")
            kb1c = cpool.tile([128, 2], F32, tag="kb1c")
            pw2c = cpool.tile([128, 2], F32, tag="pw2c")
            pw2cn = cpool.tile([128, 2], F32, tag="pw2cn")
            kw2c = cpool.tile([128, 2], F32, tag="kw2c")
            kw2cn = cpool.tile([128, 2], F32, tag="kw2cn")

            dma = nc.sync.dma_start
            mm = nc.tensor.matmul
            act = nc.scalar.activation
            V = nc.vector

            for dst, src in [
                (rT, i_rT0), (vT, i_vT0), (rbT, i_rbT0), (rfinb, i_rfin0),
                (sqA, i_sqA0), (sqBmy, i_sqBmy0),
                (Ms[0], i_mask0), (Ms[1], i_mask1),
                (mnrow, i_mnrow), (mn2row, i_mn2row), (id128, i_id128),
                (bw1a, i_bw1a), (bw1b, i_bw1b), (bw2, i_bw2), (bw3, i_bw3),
                (pw1a, i_pw1a), (pw1b, i_pw1b), (pw1Ta, i_pw1Ta), (pw1Tb, i_pw1Tb),
                (kw1a, i_kw1a), (kw1b, i_kw1b), (kw1Ta, i_kw1Ta), (kw1Tb, i_kw1Tb),
                (bb1c, i_bb1c), (bb2c, i_bb2c), (bb3c, i_bb3c),
                (pb1c, i_pb1c), (kb1c, i_kb1c),
                (pw2c, i_pw2c), (pw2cn, i_pw2cn), (kw2c, i_kw2c), (kw2cn, i_kw2cn),
            ]:
                dma(dst[:], src)
            nc.gpsimd.memset(ones_col[:], 1.0)

            def emit_flight(f):
                # PE-transpose up to 8 ORDER-consecutive j-tiles of rbT into
                # PSUM, then one wide DVE copy into the ORDER-indexed RN.
                rnp = rnpool.tile([128, 1024], F16, tag="rnp")
                lo = 8 * f
                hi = min(lo + 8, NT)
                for oi in range(lo, hi):
                    jt = ORDER[oi]
                    nc.tensor.transpose(
                        rnp[:, (oi - lo) * 128:(oi - lo + 1) * 128],
                        rbT[:, jt * 128:(jt + 1) * 128], id128[:])
                w = (hi - lo) * 128
                V.tensor_copy(RN[:, lo * 128:lo * 128 + w], rnp[:, 0:w])

            def force_pass(h, inject):
                # inject: list of stage closures from the previous half's
                # update, emitted one per order-index so PE/ACT force work
                # interleaves between dependent update stages.
                stages = list(inject) if inject else []
                RHh, Yh, Uh, Mh, GV = RHs[h], Ys[h], Us[h], Ms[h], GVs[h]
                f2p = fpool.tile([128, RHh], F32, tag=f"f2p{h}", name=f"f2p{h}")
                sp = spool.tile([1, RHh], F32, tag=f"sp{h}", name=f"sp{h}")
                for oi in range(NT + LAG):
                    if oi < NT:
                        if h == 0 and oi % 8 == 0:
                            emit_flight(oi // 8)
                        if INJ <= oi < INJ + len(stages):
                            stages[oi - INJ]()
                        jt = ORDER[oi]
                        js = slice(jt * 128, (jt + 1) * 128)
                        d2p = d2pool.tile([128, RH0], F32, tag="d2")
                        mm(d2p[:, 0:RHh], rbT[:, js], rfinb[:, HS[h]],
                           start=True, stop=False)
                        mm(d2p[:, 0:RHh], sqA[:, js], sqBmy[:, HS[h]],
                           start=False, stop=True)
                        act(Yh[:, oi * RHh:(oi + 1) * RHh], d2p[:, 0:RHh],
                            AF.Abs_reciprocal_sqrt, scale=-2.0)
                        if oi % GV == GV - 1 or oi == NT - 1:
                            g0 = (oi // GV) * GV
                            gs = slice(g0 * RHh, (oi + 1) * RHh)
                            w = (oi + 1 - g0) * RHh
                            t2 = wpool.tile([128, GV * RHh], F16, tag=f"t2g{h}")
                            V.tensor_tensor(Yh[:, gs].bitcast(U16),
                                            Yh[:, gs].bitcast(U16),
                                            Mh[:, gs], op=OP.bitwise_and)
                            V.tensor_tensor(t2[:, 0:w], Yh[:, gs], Yh[:, gs],
                                            op=OP.mult)
                            V.tensor_tensor(Uh[:, gs], t2[:, 0:w], Yh[:, gs],
                                            op=OP.mult)
                    if oi >= LAG:
                        k = oi - LAG
                        ks = slice(k * 128, (k + 1) * 128)
                        us = slice(k * RHh, (k + 1) * RHh)
                        mm(f2p[:], RN[:, ks], Uh[:, us],
                           start=(k == 0), stop=(k == NT - 1))
                        mm(sp[:], ones_col[:], Uh[:, us],
                           start=(k == 0), stop=(k == NT - 1))
                for s in stages[NT - INJ:]:
                    s()
                return f2p, sp

            def make_update(h, f2p, sp, it):
                hsl, RHh, off = HS[h], RHs[h], OFFs[h]
                last = (it == iters - 1)
                st = {}

                def s_integrate():
                    # integrate: v += a*dt ; r += v*dt; then layer-1 matmuls
                    s_sb = wpool.tile([1, RHh], F32, tag=f"s_sb{h}")
                    V.tensor_scalar_mul(s_sb[:], sp[:], -G2)
                    sbc = wpool.tile([128, RHh], F32, tag=f"sbc{h}")
                    nc.gpsimd.partition_broadcast(sbc[:], s_sb[:])
                    t1 = wpool.tile([128, RHh], F32, tag=f"t1{h}")
                    V.tensor_tensor(t1[:], sbc[:], rT[:, hsl], op=OP.mult)
                    V.scalar_tensor_tensor(t1[:], f2p[:], G2, t1[:],
                                           op0=OP.mult, op1=OP.add)
                    V.tensor_tensor(vT[:, hsl], vT[:, hsl], t1[:], op=OP.add)
                    V.scalar_tensor_tensor(rT[:, hsl], vT[:, hsl], DT,
                                           rT[:, hsl], op0=OP.mult, op1=OP.add)
                    st["rmidb"] = wpool.tile([128, RHh], F16, tag=f"rmidb{h}", name=f"rmidb{h}")
                    st["vmidb"] = wpool.tile([128, RHh], F16, tag=f"vmidb{h}", name=f"vmidb{h}")
                    V.tensor_copy(st["rmidb"][:], rT[:, hsl])
                    V.tensor_copy(st["vmidb"][:], vT[:, hsl])
                    st["hp1"] = d2pool.tile([128, RH0], F32, tag="d2", name="hp1")
                    mm(st["hp1"][0:HID, 0:RHh], bw1a[:], st["rmidb"][:],
                       start=True, stop=False)
                    mm(st["hp1"][0:HID, 0:RHh], bw1b[:], st["vmidb"][:],
                       start=False, stop=True)

                def s_layer2():
                    st["h1b"] = wpool.tile([HID, RHh], F16, tag=f"h1b{h}", name=f"h1b{h}")
                    act(st["h1b"][:], st["hp1"][0:HID, 0:RHh], AF.Tanh,
                        bias=bb1c[:])
                    st["hp2"] = d2pool.tile([128, RH0], F32, tag="d2", name="hp2")
                    mm(st["hp2"][0:HID, 0:RHh], bw2[:], st["h1b"][:],
                       start=True, stop=True)

                def s_layer3():
                    st["h2b"] = wpool.tile([HID, RHh], F16, tag=f"h2b{h}", name=f"h2b{h}")
                    act(st["h2b"][:], st["hp2"][0:HID, 0:RHh], AF.Tanh,
                        bias=bb2c[:])
                    st["hp3"] = d2pool.tile([128, RH0], F32, tag="d2", name="hp3")
                    mm(st["hp3"][:, 0:RHh], bw3[:], st["h2b"][:],
                       start=True, stop=True)

                def s_bapply():
                    tnh = wpool.tile([128, RHh], F16, tag=f"tnh{h}")
                    act(tnh[:], st["hp3"][:, 0:RHh], AF.Tanh, bias=bb3c[:])
                    V.scalar_tensor_tensor(rT[:, hsl], tnh[:], 0.1, rT[:, hsl],
                                           op0=OP.mult, op1=OP.add)

                stages = [s_integrate, s_layer2, s_layer3, s_bapply]

                if it % 5 == 0:
                    # Hamiltonian Euler step, staged: mm pair / act+grad / apply
                    ham_state = {}

                    def ham_mm(net, q):
                        def f():
                            if net == 0 and q == 0:
                                rm2b = wpool.tile([128, RHh], F16,
                                                  tag=f"rm2b{h}")
                                V.tensor_copy(rm2b[:], rT[:, hsl])
                                ham_state["rm2b"] = rm2b
                            w1s = (pw1a, pw1b) if net == 0 else (kw1a, kw1b)
                            src = ham_state["rm2b"] if net == 0 else st["vmidb"]
                            tp = d2pool.tile([128, RH0], F32, tag="d2")
                            mm(tp[:, 0:RHh], w1s[q][:], src[:],
                               start=True, stop=True)
                            ham_state[f"tp{net}{q}"] = tp
                        return f

                    def ham_grad(net, q):
                        def f():
                            bcol = pb1c if net == 0 else kb1c
                            wcol = pw2c if net == 0 else kw2c
                            wcoln = pw2cn if net == 0 else kw2cn
                            tq = wpool.tile([128, RHh], F16, tag=f"tq{h}")
                            act(tq[:], ham_state[f"tp{net}{q}"][:, 0:RHh],
                                AF.Tanh, bias=bcol[:, q:q + 1])
                            t2q = wpool.tile([128, RHh], F16, tag=f"t2q{h}")
                            V.tensor_tensor(t2q[:], tq[:], tq[:], op=OP.mult)
                            g = wpool.tile([128, RHh], F16, tag=f"gq{q}{h}")
                            V.tensor_scalar(g[:], t2q[:], wcoln[:, q:q + 1],
                                            wcol[:, q:q + 1],
                                            op0=OP.mult, op1=OP.add)
                            ham_state[f"g{net}{q}"] = g
                        return f

                    def ham_apply(net):
                        def f():
                            wTs = (pw1Ta, pw1Tb) if net == 0 else (kw1Ta, kw1Tb)
                            dst = rT if net == 0 else vT
                            coef = -0.01 if net == 0 else 0.01
                            dpp = d2pool.tile([128, RH0], F32, tag="d2")
                            for q in range(2):
                                mm(dpp[:, 0:RHh], wTs[q][:],
                                   ham_state[f"g{net}{q}"][:],
                                   start=(q == 0), stop=(q == 1))
                            V.scalar_tensor_tensor(dst[:, hsl], dpp[:, 0:RHh],
                                                   coef, dst[:, hsl],
                                                   op0=OP.mult, op1=OP.add)
                        return f

                    for net in range(2):
                        stages += [ham_mm(net, 0), ham_grad(net, 0),
                                   ham_mm(net, 1), ham_grad(net, 1),
                                   ham_apply(net)]

                def s_damp_sq():
                    V.tensor_scalar_mul(vT[:, hsl], vT[:, hsl], DAMP)
                    r2b = wpool.tile([128, RHh], F16, tag=f"r2b{h}")
                    act(r2b[:], rT[:, hsl], AF.Square)
                    st["sqp"] = spool.tile([1, RHh], F32, tag=f"sp{h}",
                                           name=f"sq{h}")
                    mm(st["sqp"][:], ones_col[:], r2b[:], start=True, stop=True)

                def s_clip():
                    sqp = st["sqp"]
                    frinv = wpool.tile([1, RHh], F32, tag=f"frinv{h}")
                    act(frinv[:], sqp[:], AF.Abs_reciprocal_sqrt)
                    sqf = wpool.tile([1, RHh], F32, tag=f"sqf{h}")
                    V.tensor_copy(sqf[:], sqp[:])
                    fr = wpool.tile([1, RHh], F32, tag=f"fr{h}")
                    V.tensor_tensor(fr[:], frinv[:], mnrow[:, hsl], op=OP.mult)
                    V.tensor_scalar_min(fr[:], fr[:], 1.0)
                    fbc = wpool.tile([128, RHh], F32, tag=f"fbc{h}")
                    nc.gpsimd.partition_broadcast(fbc[:], fr[:])
                    V.tensor_tensor(rT[:, hsl], rT[:, hsl], fbc[:], op=OP.mult)
                    sqpost = wpool.tile([1, RHh], F32, tag=f"sqpost{h}")
                    V.tensor_tensor(sqpost[:], sqf[:], mn2row[:, hsl],
                                    op=OP.min)
                    V.tensor_scalar_mul(sqBmy[0:1, hsl], sqpost[:], -0.5)
                    V.tensor_copy(rfinb[:, hsl], rT[:, hsl])

                def s_gather():
                    blob = dpool.tile([129, RHh], F16, tag=f"blob{h}")
                    dma(blob[0:128, :], rfinb[:, hsl])
                    dma(blob[128:129, :], sqBmy[0:1, hsl])
                    st["gath"] = dpool.tile([NC * 129, RHh], F16, tag=f"gath{h}",
                                            name=f"gath{h}", addr_space="Shared")
                    nc.gpsimd.collective_compute(
                        "AllGather", OP.bypass,
                        replica_groups=[list(range(NC))],
                        ins=[blob[:].opt()], outs=[st["gath"][:].opt()],
                    )

                def s_unpack():
                    gath = st["gath"]
                    rq = [nc.sync, nc.gpsimd] * 4
                    for g in range(NC):
                        rq[g].dma_start(
                            rbT[:, g * R + off:g * R + off + RHh],
                            gath[g * 129:g * 129 + 128, :])
                        nc.gpsimd.dma_start(
                            sqA[1:2, g * R + off:g * R + off + RHh],
                            gath[g * 129 + 128:g * 129 + 129, :])

                stages += [s_damp_sq, s_clip]
                if not last:
                    stages += [s_gather, s_unpack]
                return stages

            pending = None
            for it in range(iters):
                f0, s0 = force_pass(0, inject=pending)
                f1, s1 = force_pass(1, inject=make_update(0, f0, s0, it))
                pending = make_update(1, f1, s1, it)
            for s in pending:
                s()
            dma(o_r, rT[:])

    nc.compile()
    return nc


def _prep_inputs(x, attractor_positions, attractor_masses, init_velocities,
                 pw1, pb1, pw2, pb2, kw1, kb1, kw2, kb2,
                 bw1, bb1, bw2, bb2, bw3, bb3, fw, fb):
    f16 = np.float16
    parts = np.asarray(x, np.float32).reshape(-1, D)
    ap = np.asarray(attractor_positions, np.float32)
    v0in = np.asarray(init_velocities, np.float32)
    r0 = np.concatenate([parts, np.tile(ap, (B, 1))], 0)
    v0 = np.concatenate([np.zeros_like(parts), np.tile(v0in, (B, 1))], 0)
    npad = MP - M0
    pad_pos = np.zeros((npad, D), np.float32)
    for p in range(npad):
        pad_pos[p, p % D] = 60.0 + 0.5 * p
    r0p = np.concatenate([r0, pad_pos], 0)
    v0p = np.concatenate([v0, np.zeros((npad, D), np.float32)], 0)
    mn_vec = np.concatenate([np.full((M0,), MAXNORM, np.float32),
                             np.full((npad,), 1e30, np.float32)])
    mn2_vec = np.concatenate([np.full((M0,), MAXNORM * MAXNORM, np.float32),
                              np.full((npad,), 1e30, np.float32)])

    # kill mask: diagonal + duplicated-attractor pairs + zero-mass pad rows
    kill = np.zeros((MP, MP), dtype=bool)
    kill[np.arange(MP), np.arange(MP)] = True
    kill[M0:, :] = True
    base = B * N
    for k in range(K):
        idxs = [base + b * K + k for b in range(B)]
        for a_ in idxs:
            for b_ in idxs:
                if a_ != b_:
                    kill[a_, b_] = True

    sq0 = np.sum(r0p * r0p, -1).astype(np.float32)
    rbT0 = r0p.T.astype(f16)                      # [128, MP]
    sqA0 = np.stack([np.ones((MP,), np.float32),
                     -0.5 * sq0], 0).astype(f16)  # [2, MP]

    w = {
        "id128": np.eye(128, dtype=f16),
        "bw1a": np.asarray(bw1, np.float32)[0:128].astype(f16),
        "bw1b": np.asarray(bw1, np.float32)[128:256].astype(f16),
        "bw2": np.asarray(bw2, np.float32).astype(f16),
        "bw3": np.asarray(bw3, np.float32).astype(f16),
        "pw1a": np.asarray(pw1, np.float32)[:, 0:128].astype(f16),
        "pw1b": np.asarray(pw1, np.float32)[:, 128:256].astype(f16),
        "pw1Ta": np.ascontiguousarray(np.asarray(pw1, np.float32).T[0:128]).astype(f16),
        "pw1Tb": np.ascontiguousarray(np.asarray(pw1, np.float32).T[128:256]).astype(f16),
        "kw1a": np.asarray(kw1, np.float32)[:, 0:128].astype(f16),
        "kw1b": np.asarray(kw1, np.float32)[:, 128:256].astype(f16),
        "kw1Ta": np.ascontiguousarray(np.asarray(kw1, np.float32).T[0:128]).astype(f16),
        "kw1Tb": np.ascontiguousarray(np.asarray(kw1, np.float32).T[128:256]).astype(f16),
        "bb1c": np.asarray(bb1, np.float32).reshape(HID, 1),
        "bb2c": np.asarray(bb2, np.float32).reshape(HID, 1),
        "bb3c": np.asarray(bb3, np.float32).reshape(128, 1),
        "pb1c": np.asarray(pb1, np.float32).reshape(2, 128).T.copy(),
        "kb1c": np.asarray(kb1, np.float32).reshape(2, 128).T.copy(),
        "pw2c": np.asarray(pw2, np.float32).reshape(2, 128).T.copy(),
        "pw2cn": (-np.asarray(pw2, np.float32)).reshape(2, 128).T.copy(),
        "kw2c": np.asarray(kw2, np.float32).reshape(2, 128).T.copy(),
        "kw2cn": (-np.asarray(kw2, np.float32)).reshape(2, 128).T.copy(),
    }

    in_maps = []
    for c in range(NC):
        rows = slice(c * R, (c + 1) * R)
        m = dict(w)
        m["rT0"] = np.ascontiguousarray(r0p[rows].T)
        m["vT0"] = np.ascontiguousarray(v0p[rows].T)
        m["rbT0"] = rbT0
        m["rfin0"] = np.ascontiguousarray(rbT0[:, rows])
        m["sqA0"] = sqA0
        m["sqBmy0"] = np.stack(
            [-0.5 * sq0[rows], np.ones((R,), np.float32)], 0).astype(f16)
        kl = np.where(kill[:, rows], np.uint16(0), np.uint16(0xFFFF))  # [MP, R]
        for h, (lo, hi) in enumerate([(0, RH0), (RH0, R)]):
            mh = np.concatenate(
                [kl[jt * 128:(jt + 1) * 128, lo:hi] for jt in ORDER], axis=1)
            m[f"mask{h}"] = np.ascontiguousarray(mh)
        m["mnrow"] = mn_vec[rows].reshape(1, R)
        m["mn2row"] = mn2_vec[rows].reshape(1, R)
        in_maps.append(m)
    return in_maps


def _epilogue(r_fin, attractor_masses, fw, fb):
    fp = r_fin[:B * N].reshape(B, N, D)
    fa = r_fin[B * N:M0].reshape(B, K, D)
    am = np.asarray(attractor_masses, np.float32)
    pm = np.ones((B, N), np.float32)
    am2 = np.broadcast_to(am[None, :], (B, K))
    allm = np.concatenate([pm, am2], 1)
    allp = np.concatenate([fp, fa], 1)
    tot = allm.sum(1, keepdims=True)
    com = (allm[..., None] * allp).sum(1) / tot
    centered = allp - com[:, None, :]
    var = (centered ** 2).mean(1)
    skew = (centered ** 3).mean(1)
    p0 = allp[0]
    sq0 = np.sum(p0 * p0, -1)
    dmat = np.sqrt(np.maximum(sq0[:, None] + sq0[None, :] - 2.0 * p0 @ p0.T, 0.0))
    iu, ju = np.triu_indices(p0.shape[0], 1)
    dd = dmat[iu, ju]
    st = np.stack([dd.mean(), dd.std(ddof=1), dd.min(), dd.max()])
    stb = np.broadcast_to(st[None, :], (B, 4)).astype(np.float32)
    feat = np.concatenate([com, var, skew, stb], -1)
    return (feat @ np.asarray(fw, np.float32) + np.asarray(fb, np.float32)).astype(np.float32)


def _wire_ntff_hook():
    import types
    try:
        import antenv.axon_hooks  # noqa: F401
        return True
    except ImportError:
        pass
    try:
        import antenv
        from trn_agent_boot.trn_boot import _ntff_profile_via_ctypes
        mod = types.ModuleType("antenv.axon_hooks")
        _h = [None]
        mod.set_axon_ntff_profile_hook = lambda h: _h.__setitem__(0, h)
        mod.get_axon_ntff_profile_hook = lambda: _h[0]
        sys.modules["antenv.axon_hooks"] = mod
        antenv.axon_hooks = mod
        mod.set_axon_ntff_profile_hook(
            _ntff_profile_via_ctypes("/opt/axon/libaxon_pjrt.so"))
        return True
    except Exception as e:
        print(f"ntff hook wiring failed ({e}); running without trace")
        return False


def kernel(**inputs):
    from concourse.bass_utils import run_bass_kernel_spmd

    iters = int(os.environ.get("KERNEL_ITERS", ITERS))
    trace = bool(int(os.environ.get("KERNEL_TRACE", "0")))
    if trace:
        trace = _wire_ntff_hook()
    if iters not in _PROGRAM_CACHE:
        _PROGRAM_CACHE[iters] = _build_program(iters)
    nc = _PROGRAM_CACHE[iters]
    in_maps = _prep_inputs(**inputs)
    res = run_bass_kernel_spmd(nc, in_maps, core_ids=list(range(NC)), trace=trace)
    if trace and res.exec_time_ns is not None:
        print(f"HW exec time: {res.exec_time_ns} ns")
    rT_full = np.zeros((128, MP), np.float32)
    for c in range(NC):
        rT_full[:, c * R:(c + 1) * R] = res.results[c]["r_out"]
    r_fin = np.ascontiguousarray(rT_full.T)[:M0]
    out = _epilogue(r_fin, inputs["attractor_masses"], inputs["fw"], inputs["fb"])
    kernel.last_exec_time_ns = res.exec_time_ns if trace else None
    return out


if __name__ == "__main__":
    rng = np.random.default_rng(0)
    demo = {
        "x": rng.standard_normal((B, N, D)).astype(np.float32),
        "attractor_positions": rng.standard_normal((K, D)).astype(np.float32),
        "attractor_masses": np.full((K,), 0.1, np.float32),
        "init_velocities": np.zeros((K, D), np.float32),
        "pw1": rng.standard_normal((D, 2 * D)).astype(np.float32) / 16,
        "pb1": np.zeros((2 * D,), np.float32),
        "pw2": rng.standard_normal((2 * D, 1)).astype(np.float32) / 16,
        "pb2": np.zeros((1,), np.float32),
        "kw1": rng.standard_normal((D, 2 * D)).astype(np.float32) / 16,
        "kb1": np.zeros((2 * D,), np.float32),
        "kw2": rng.standard_normal((2 * D, 1)).astype(np.float32) / 16,
        "kb2": np.zeros((1,), np.float32),
        "bw1": rng.standard_normal((2 * D, HID)).astype(np.float32) / 16,
        "bb1": np.zeros((HID,), np.float32),
        "bw2": rng.standard_normal((HID, HID)).astype(np.float32) / 8,
        "bb2": np.zeros((HID,), np.float32),
        "bw3": rng.standard_normal((HID, D)).astype(np.float32) / 8,
        "bb3": np.zeros((D,), np.float32),
        "fw": rng.standard_normal((3 * D + 4, D)).astype(np.float32) / 20,
        "fb": np.zeros((D,), np.float32),
    }
    out = kernel(**demo)
    print("out", out.shape, np.abs(out).max())
